# revision 20
# baseline (speedup 1.0000x reference)
"""Trainium2 Bass kernel for CGCalculatorSingle (segment_reduce).

Computes out[b,f,mu[k]] += C[k] * X1[b,f,m1[k]] * X2[b,f,m2[k]] for k in [0,NNZ).

Strategy:
- Pure data parallel over the batch (environments) axis: 8 NeuronCores, 500 envs each.
- Per core, the shard is viewed as [128 partitions, 5500 free] fp32 where each
  partition holds 500 contiguous (env,f) rows of 11 m-values -> fully contiguous
  per-partition DMA (near-peak HBM bandwidth).
- The index/coefficient buffers are tiny and known at kernel-build time, so the
  gather/scatter pattern is specialized into the instruction stream: products of
  deduplicated (m1,m2) column pairs (stride-11 APs) on VectorE, fused
  scale+accumulate via scalar_tensor_tensor into the output columns.
"""

import numpy as np
from contextlib import ExitStack

B, F, M = 4000, 128, 11
NCORES = 8
BS = B // NCORES            # 500 envs per core
PART = 128
FREE = BS * F * M // PART   # 5500 fp32 per partition
ROWS = FREE // M            # 500 rows per partition


def _build_plan(m1, m2, mu, C):
    """Group NNZ entries into deduped (a,b)->[(j,c)...] pairs, plus merged
    weighted-sum groups.

    Returns (pairs, merges):
    - pairs: {(a, b): [(j, c), ...]} for entries evaluated as products of
      single columns.
    - merges: [(side, other_m, j, [(m_i, c_i), ...])]: entries sharing one
      (X2-column b, output j) — side 0 — or (X1-column a, j) — side 1 — whose
      column pair is used nowhere else. Evaluated as
      out_j += (sum_i c_i * Xs_{m_i}) * Xo_{other_m}, which costs k+1 ops
      instead of 2k.
    """
    triples = {}
    for a, b, j, c in zip(m1.tolist(), m2.tolist(), mu.tolist(), C.tolist()):
        key = (int(a), int(b), int(j))
        triples[key] = triples.get(key, 0.0) + float(c)
    triples = {k: c for k, c in triples.items() if c != 0.0}

    pair_count = {}
    for a, b, j in triples:
        pair_count[(a, b)] = pair_count.get((a, b), 0) + 1

    by_bj = {}
    by_aj = {}
    for (a, b, j), c in triples.items():
        if pair_count[(a, b)] == 1:
            by_bj.setdefault((b, j), []).append((a, c))
            by_aj.setdefault((a, j), []).append((b, c))

    merges = []
    consumed = set()
    # Greedily take larger groups first, alternating sides for fairness.
    cands = [(len(v), 0, bj, v) for bj, v in by_bj.items() if len(v) >= 2]
    cands += [(len(v), 1, aj, v) for aj, v in by_aj.items() if len(v) >= 2]
    cands.sort(key=lambda t: -t[0])
    for _, side, (om, j), entries in cands:
        avail = [
            (m, c)
            for m, c in entries
            if ((m, om) if side == 0 else (om, m)) not in consumed
        ]
        if len(avail) < 2:
            continue
        for m, _ in avail:
            consumed.add((m, om) if side == 0 else (om, m))
        merges.append((side, om, j, avail))

    pairs = {}
    for (a, b, j), c in triples.items():
        if (a, b) in consumed:
            continue
        pairs.setdefault((a, b), []).append((j, c))
    return pairs, merges


UNIT_STRIDE_TEST = False


def _emit_compute(nc, mybir, x1f, x2f, accf, scratch_pool, pairs, merges, vec):
    """Emit the DVE op schedule.

    x1f/x2f/accf: [128, FREE] flat APs of the SBUF tiles.
    Returns nothing; accf holds the final result columns.
    """
    mult = mybir.AluOpType.mult
    add = mybir.AluOpType.add

    x1v = x1f.rearrange("p (r m) -> p r m", m=M)
    x2v = x2f.rearrange("p (r m) -> p r m", m=M)
    accv = accf.rearrange("p (r m) -> p r m", m=M)

    init = [False] * M
    done = set()

    if UNIT_STRIDE_TEST:
        # Timing experiment only (wrong math): treat tiles as m-major so every
        # op is unit-stride with identical op/element counts.
        flat = {id(x1v): x1f, id(x2v): x2f, id(accv): accf}

        def col(v, m):
            return flat[id(v)][:, m * ROWS : (m + 1) * ROWS]
    else:
        def col(v, m):
            return v[:, :, m]

    # Phase 0: merged groups — entries sharing (other-column, j) evaluate as
    # out_j += (sum_i c_i * Xs_{m_i}) * Xo_{om} via a weighted-sum chain.
    for side, om, j, entries in merges:
        sv = x1v if side == 0 else x2v
        other = col(x2v if side == 0 else x1v, om)
        s_ap = None
        for i in range(len(entries) - 1):
            m_i, c_i = entries[i]
            m_n, c_n = entries[i + 1]
            src = col(sv, m_i) if s_ap is None else s_ap
            s_tile = scratch_pool.tile([PART, ROWS], mybir.dt.float32, tag="msum")
            vec.scalar_tensor_tensor(
                s_tile[:], src, c_i / c_n, col(sv, m_n), op0=mult, op1=add
            )
            s_ap = s_tile[:]
        c_last = entries[-1][1]
        if not init[j]:
            vec.scalar_tensor_tensor(
                col(accv, j), s_ap, c_last, other, op0=mult, op1=mult
            )
            init[j] = True
        else:
            p_tile = scratch_pool.tile([PART, ROWS], mybir.dt.float32, tag="msum")
            vec.tensor_tensor(p_tile[:], s_ap, other, op=mult)
            vec.scalar_tensor_tensor(
                col(accv, j), p_tile[:], c_last, col(accv, j), op0=mult, op1=add
            )

    # Phase 1: for each j, bootstrap its accumulator column with a fully fused
    # op: acc[:, :, j] = (X1_a * c) * X2_b. The pair's remaining j's are then
    # served by rescaling that column before anything else touches it.
    for (a, b), jlist in pairs.items():
        j0, c0 = jlist[0]
        if init[j0] or (len(jlist) > 1 and c0 == 0.0):
            continue
        vec.scalar_tensor_tensor(
            col(accv, j0), col(x1v, a), c0, col(x2v, b), op0=mult, op1=mult
        )
        init[j0] = True
        for j, c in jlist[1:]:
            r = c / c0
            if init[j]:
                vec.scalar_tensor_tensor(
                    col(accv, j), col(accv, j0), r, col(accv, j), op0=mult, op1=add
                )
            else:
                # Single-source scaled copy: ScalarE has its own SBUF ports and
                # runs concurrently with the VectorE stream.
                nc.scalar.mul(col(accv, j), col(accv, j0), r)
                init[j] = True
        done.add((a, b))

    # Phase 2: remaining pairs, grouped so pairs sharing one X2 column with a
    # contiguous run of m1 values fuse into one wide product op:
    #   prod[:, 0:g*ROWS] = X1[:, :, a0:a0+g] * X2[:, :, b] (b broadcast via a
    # step-0 AP). This amortizes the per-op fixed cost over g columns.
    rest = sorted(p for p in pairs if p not in done)
    by_b = {}
    for a, b in rest:
        by_b.setdefault(b, []).append(a)
    runs = []  # (a0, b, g)
    for b, alist in by_b.items():
        alist.sort()
        a0 = prev = alist[0]
        for a in alist[1:]:
            if a == prev + 1:
                prev = a
                continue
            runs.append((a0, b, prev - a0 + 1))
            a0 = prev = a
        runs.append((a0, b, prev - a0 + 1))

    for a0, b, g in runs:
        # Product block laid out r-major: prod[p, r*g + gi] = X1[p,r,a0+gi]*X2[p,r,b].
        # The op iterates gi innermost: X1 reads are contiguous runs of g
        # elements, X2 re-reads one element (step-0 broadcast).
        prod = scratch_pool.tile([PART, g * ROWS], mybir.dt.float32, tag="prod")
        prodv = prod[:].rearrange("p (r g) -> p r g", g=g)
        if UNIT_STRIDE_TEST:
            in0 = x1f[:, a0 * ROWS : (a0 + g) * ROWS].rearrange(
                "p (r g) -> p r g", g=g
            )
            in1 = (
                x2f[:, b * ROWS : (b + 1) * ROWS]
                .rearrange("p (r g) -> p r g", g=1)
                .broadcast_to([PART, ROWS, g])
            )
        else:
            in0 = x1v[:, :, a0 : a0 + g]
            in1 = x2v[:, :, b : b + 1].broadcast_to([PART, ROWS, g])
        vec.tensor_tensor(prodv, in0, in1, op=mult)
        for gi in range(g):
            a = a0 + gi
            for j, c in pairs[(a, b)]:
                if init[j]:
                    vec.scalar_tensor_tensor(
                        col(accv, j), prodv[:, :, gi], c, col(accv, j),
                        op0=mult, op1=add,
                    )
                else:
                    nc.scalar.mul(col(accv, j), prodv[:, :, gi], c)
                    init[j] = True

    # Phase 3: zero any output column no entry maps to.
    for j in range(M):
        if not init[j]:
            vec.memset(col(accv, j), 0.0)


def _build_program(plan, repeat=1):
    pairs, merges = plan
    import concourse.bass as bass
    import concourse.tile as tile
    from concourse import bacc, mybir

    nc = bacc.Bacc(
        "TRN2",
        target_bir_lowering=False,
        debug=False,
        enable_asserts=True,
        num_devices=NCORES,
    )
    x1_d = nc.dram_tensor("x1", [PART, FREE], mybir.dt.float32, kind="ExternalInput").ap()
    x2_d = nc.dram_tensor("x2", [PART, FREE], mybir.dt.float32, kind="ExternalInput").ap()
    out_d = nc.dram_tensor("out", [PART, FREE], mybir.dt.float32, kind="ExternalOutput").ap()

    with ExitStack() as ctx:
        tc = ctx.enter_context(tile.TileContext(nc))
        io_pool = ctx.enter_context(tc.tile_pool(name="io", bufs=1))
        scratch_pool = ctx.enter_context(tc.tile_pool(name="scratch", bufs=2))

        x1t = io_pool.tile([PART, FREE], mybir.dt.float32)
        nc.sync.dma_start(x1t[:], x1_d)
        x2t = io_pool.tile([PART, FREE], mybir.dt.float32)
        nc.sync.dma_start(x2t[:], x2_d)
        acct = io_pool.tile([PART, FREE], mybir.dt.float32)

        for _ in range(repeat):
            _emit_compute(
                nc, mybir, x1t[:], x2t[:], acct[:], scratch_pool, pairs, merges,
                nc.vector,
            )

        nc.sync.dma_start(out_d, acct[:])

    nc.compile()
    return nc


TRACE = False
LAST_EXEC_NS = None
LAST_TRACE_DIR = None


def kernel(X1, X2, m1, m2, mu, C):
    global LAST_EXEC_NS, LAST_TRACE_DIR
    from concourse.bass_utils import run_bass_kernel_spmd

    X1 = np.ascontiguousarray(np.asarray(X1, dtype=np.float32))
    X2 = np.ascontiguousarray(np.asarray(X2, dtype=np.float32))
    plan = _build_plan(np.asarray(m1), np.asarray(m2), np.asarray(mu), np.asarray(C))

    nc = _build_program(plan)

    in_maps = []
    for i in range(NCORES):
        sl = slice(i * BS, (i + 1) * BS)
        in_maps.append(
            {
                "x1": X1[sl].reshape(PART, FREE),
                "x2": X2[sl].reshape(PART, FREE),
            }
        )

    kwargs = {}
    if TRACE:
        import tempfile

        LAST_TRACE_DIR = tempfile.mkdtemp(prefix="bass_trace_")
        kwargs = dict(trace=True, tmpdir=LAST_TRACE_DIR)
    res = run_bass_kernel_spmd(nc, in_maps, list(range(NCORES)), **kwargs)
    LAST_EXEC_NS = res.exec_time_ns
    shards = [res.results[i]["out"].reshape(BS, F, M) for i in range(NCORES)]
    return np.concatenate(shards, axis=0)


# revision 21
# speedup vs baseline: 1.0221x; 1.0221x over previous
"""Trainium2 Bass kernel for CGCalculatorSingle (segment_reduce).

Computes out[b,f,mu[k]] += C[k] * X1[b,f,m1[k]] * X2[b,f,m2[k]] for k in [0,NNZ).

Strategy:
- Pure data parallel over the batch (environments) axis: 8 NeuronCores, 500 envs each.
- Per core, the shard is viewed as [128 partitions, 5500 free] fp32 where each
  partition holds 500 contiguous (env,f) rows of 11 m-values -> fully contiguous
  per-partition DMA (near-peak HBM bandwidth).
- The index/coefficient buffers are tiny and known at kernel-build time, so the
  gather/scatter pattern is specialized into the instruction stream: products of
  deduplicated (m1,m2) column pairs (stride-11 APs) on VectorE, fused
  scale+accumulate via scalar_tensor_tensor into the output columns.
"""

import numpy as np
from contextlib import ExitStack

B, F, M = 4000, 128, 11
NCORES = 8
BS = B // NCORES            # 500 envs per core
PART = 128
FREE = BS * F * M // PART   # 5500 fp32 per partition
ROWS = FREE // M            # 500 rows per partition


def _build_plan(m1, m2, mu, C):
    """Group NNZ entries into deduped (a,b)->[(j,c)...] pairs, plus merged
    weighted-sum groups.

    Returns (pairs, merges):
    - pairs: {(a, b): [(j, c), ...]} for entries evaluated as products of
      single columns.
    - merges: [(side, other_m, j, [(m_i, c_i), ...])]: entries sharing one
      (X2-column b, output j) — side 0 — or (X1-column a, j) — side 1 — whose
      column pair is used nowhere else. Evaluated as
      out_j += (sum_i c_i * Xs_{m_i}) * Xo_{other_m}, which costs k+1 ops
      instead of 2k.
    """
    triples = {}
    for a, b, j, c in zip(m1.tolist(), m2.tolist(), mu.tolist(), C.tolist()):
        key = (int(a), int(b), int(j))
        triples[key] = triples.get(key, 0.0) + float(c)
    triples = {k: c for k, c in triples.items() if c != 0.0}

    pair_count = {}
    for a, b, j in triples:
        pair_count[(a, b)] = pair_count.get((a, b), 0) + 1

    by_bj = {}
    by_aj = {}
    for (a, b, j), c in triples.items():
        if pair_count[(a, b)] == 1:
            by_bj.setdefault((b, j), []).append((a, c))
            by_aj.setdefault((a, j), []).append((b, c))

    merges = []
    consumed = set()
    # Greedily take larger groups first, alternating sides for fairness.
    cands = [(len(v), 0, bj, v) for bj, v in by_bj.items() if len(v) >= 2]
    cands += [(len(v), 1, aj, v) for aj, v in by_aj.items() if len(v) >= 2]
    cands.sort(key=lambda t: -t[0])
    for _, side, (om, j), entries in cands:
        avail = [
            (m, c)
            for m, c in entries
            if ((m, om) if side == 0 else (om, m)) not in consumed
        ]
        if len(avail) < 2:
            continue
        for m, _ in avail:
            consumed.add((m, om) if side == 0 else (om, m))
        merges.append((side, om, j, avail))

    pairs = {}
    for (a, b, j), c in triples.items():
        if (a, b) in consumed:
            continue
        pairs.setdefault((a, b), []).append((j, c))
    return pairs, merges


UNIT_STRIDE_TEST = False


def _emit_compute(nc, mybir, x1f, x2f, accf, scratch_pool, pairs, merges, vec):
    """Emit the DVE op schedule.

    x1f/x2f/accf: [128, FREE] flat APs of the SBUF tiles.
    Returns nothing; accf holds the final result columns.
    """
    mult = mybir.AluOpType.mult
    add = mybir.AluOpType.add

    x1v = x1f.rearrange("p (r m) -> p r m", m=M)
    x2v = x2f.rearrange("p (r m) -> p r m", m=M)
    accv = accf.rearrange("p (r m) -> p r m", m=M)

    init = [False] * M
    done = set()

    if UNIT_STRIDE_TEST:
        # Timing experiment only (wrong math): treat tiles as m-major so every
        # op is unit-stride with identical op/element counts.
        flat = {id(x1v): x1f, id(x2v): x2f, id(accv): accf}

        def col(v, m):
            return flat[id(v)][:, m * ROWS : (m + 1) * ROWS]
    else:
        def col(v, m):
            return v[:, :, m]

    # Phase 0: merged groups — entries sharing (other-column, j) evaluate as
    # out_j += (sum_i c_i * Xs_{m_i}) * Xo_{om} via a weighted-sum chain.
    # Chain ops read only one input tensor, so all side-0 chains (X1-only) are
    # emitted first: they execute while the X2 DMA is still in flight (the X1
    # load finishes ~8us earlier), hiding the second load under compute.
    chain_results = {}
    for mi, (side, om, j, entries) in enumerate(merges):
        if side != 0:
            continue
        s_ap = None
        for i in range(len(entries) - 1):
            m_i, c_i = entries[i]
            m_n, c_n = entries[i + 1]
            src = col(x1v, m_i) if s_ap is None else s_ap
            s_tile = scratch_pool.tile(
                [PART, ROWS], mybir.dt.float32, tag=f"msum{mi}"
            )
            vec.scalar_tensor_tensor(
                s_tile[:], src, c_i / c_n, col(x1v, m_n), op0=mult, op1=add
            )
            s_ap = s_tile[:]
        chain_results[mi] = s_ap

    for mi, (side, om, j, entries) in enumerate(merges):
        sv = x1v if side == 0 else x2v
        other = col(x2v if side == 0 else x1v, om)
        s_ap = chain_results.get(mi)
        if s_ap is None:
            for i in range(len(entries) - 1):
                m_i, c_i = entries[i]
                m_n, c_n = entries[i + 1]
                src = col(sv, m_i) if s_ap is None else s_ap
                s_tile = scratch_pool.tile(
                    [PART, ROWS], mybir.dt.float32, tag="msum"
                )
                vec.scalar_tensor_tensor(
                    s_tile[:], src, c_i / c_n, col(sv, m_n), op0=mult, op1=add
                )
                s_ap = s_tile[:]
        c_last = entries[-1][1]
        if not init[j]:
            vec.scalar_tensor_tensor(
                col(accv, j), s_ap, c_last, other, op0=mult, op1=mult
            )
            init[j] = True
        else:
            p_tile = scratch_pool.tile([PART, ROWS], mybir.dt.float32, tag="msum")
            vec.tensor_tensor(p_tile[:], s_ap, other, op=mult)
            vec.scalar_tensor_tensor(
                col(accv, j), p_tile[:], c_last, col(accv, j), op0=mult, op1=add
            )

    # Phase 1: for each j, bootstrap its accumulator column with a fully fused
    # op: acc[:, :, j] = (X1_a * c) * X2_b. The pair's remaining j's are then
    # served by rescaling that column before anything else touches it.
    for (a, b), jlist in pairs.items():
        j0, c0 = jlist[0]
        if init[j0] or (len(jlist) > 1 and c0 == 0.0):
            continue
        vec.scalar_tensor_tensor(
            col(accv, j0), col(x1v, a), c0, col(x2v, b), op0=mult, op1=mult
        )
        init[j0] = True
        for j, c in jlist[1:]:
            r = c / c0
            if init[j]:
                vec.scalar_tensor_tensor(
                    col(accv, j), col(accv, j0), r, col(accv, j), op0=mult, op1=add
                )
            else:
                # Single-source scaled copy: ScalarE has its own SBUF ports and
                # runs concurrently with the VectorE stream.
                nc.scalar.mul(col(accv, j), col(accv, j0), r)
                init[j] = True
        done.add((a, b))

    # Phase 2: remaining pairs, grouped so pairs sharing one X2 column with a
    # contiguous run of m1 values fuse into one wide product op:
    #   prod[:, 0:g*ROWS] = X1[:, :, a0:a0+g] * X2[:, :, b] (b broadcast via a
    # step-0 AP). This amortizes the per-op fixed cost over g columns.
    rest = sorted(p for p in pairs if p not in done)
    by_b = {}
    for a, b in rest:
        by_b.setdefault(b, []).append(a)
    runs = []  # (a0, b, g)
    for b, alist in by_b.items():
        alist.sort()
        a0 = prev = alist[0]
        for a in alist[1:]:
            if a == prev + 1:
                prev = a
                continue
            runs.append((a0, b, prev - a0 + 1))
            a0 = prev = a
        runs.append((a0, b, prev - a0 + 1))

    for a0, b, g in runs:
        # Product block laid out r-major: prod[p, r*g + gi] = X1[p,r,a0+gi]*X2[p,r,b].
        # The op iterates gi innermost: X1 reads are contiguous runs of g
        # elements, X2 re-reads one element (step-0 broadcast).
        prod = scratch_pool.tile([PART, g * ROWS], mybir.dt.float32, tag="prod")
        prodv = prod[:].rearrange("p (r g) -> p r g", g=g)
        if UNIT_STRIDE_TEST:
            in0 = x1f[:, a0 * ROWS : (a0 + g) * ROWS].rearrange(
                "p (r g) -> p r g", g=g
            )
            in1 = (
                x2f[:, b * ROWS : (b + 1) * ROWS]
                .rearrange("p (r g) -> p r g", g=1)
                .broadcast_to([PART, ROWS, g])
            )
        else:
            in0 = x1v[:, :, a0 : a0 + g]
            in1 = x2v[:, :, b : b + 1].broadcast_to([PART, ROWS, g])
        vec.tensor_tensor(prodv, in0, in1, op=mult)
        for gi in range(g):
            a = a0 + gi
            for j, c in pairs[(a, b)]:
                if init[j]:
                    vec.scalar_tensor_tensor(
                        col(accv, j), prodv[:, :, gi], c, col(accv, j),
                        op0=mult, op1=add,
                    )
                else:
                    nc.scalar.mul(col(accv, j), prodv[:, :, gi], c)
                    init[j] = True

    # Phase 3: zero any output column no entry maps to.
    for j in range(M):
        if not init[j]:
            vec.memset(col(accv, j), 0.0)


def _build_program(plan, repeat=1):
    pairs, merges = plan
    import concourse.bass as bass
    import concourse.tile as tile
    from concourse import bacc, mybir

    nc = bacc.Bacc(
        "TRN2",
        target_bir_lowering=False,
        debug=False,
        enable_asserts=True,
        num_devices=NCORES,
    )
    x1_d = nc.dram_tensor("x1", [PART, FREE], mybir.dt.float32, kind="ExternalInput").ap()
    x2_d = nc.dram_tensor("x2", [PART, FREE], mybir.dt.float32, kind="ExternalInput").ap()
    out_d = nc.dram_tensor("out", [PART, FREE], mybir.dt.float32, kind="ExternalOutput").ap()

    with ExitStack() as ctx:
        tc = ctx.enter_context(tile.TileContext(nc))
        io_pool = ctx.enter_context(tc.tile_pool(name="io", bufs=1))
        scratch_pool = ctx.enter_context(tc.tile_pool(name="scratch", bufs=2))

        x1t = io_pool.tile([PART, FREE], mybir.dt.float32)
        nc.sync.dma_start(x1t[:], x1_d)
        x2t = io_pool.tile([PART, FREE], mybir.dt.float32)
        nc.sync.dma_start(x2t[:], x2_d)
        acct = io_pool.tile([PART, FREE], mybir.dt.float32)

        for _ in range(repeat):
            _emit_compute(
                nc, mybir, x1t[:], x2t[:], acct[:], scratch_pool, pairs, merges,
                nc.vector,
            )

        nc.sync.dma_start(out_d, acct[:])

    nc.compile()
    return nc


TRACE = False
LAST_EXEC_NS = None
LAST_TRACE_DIR = None


def kernel(X1, X2, m1, m2, mu, C):
    global LAST_EXEC_NS, LAST_TRACE_DIR
    from concourse.bass_utils import run_bass_kernel_spmd

    X1 = np.ascontiguousarray(np.asarray(X1, dtype=np.float32))
    X2 = np.ascontiguousarray(np.asarray(X2, dtype=np.float32))
    plan = _build_plan(np.asarray(m1), np.asarray(m2), np.asarray(mu), np.asarray(C))

    nc = _build_program(plan)

    in_maps = []
    for i in range(NCORES):
        sl = slice(i * BS, (i + 1) * BS)
        in_maps.append(
            {
                "x1": X1[sl].reshape(PART, FREE),
                "x2": X2[sl].reshape(PART, FREE),
            }
        )

    kwargs = {}
    if TRACE:
        import tempfile

        LAST_TRACE_DIR = tempfile.mkdtemp(prefix="bass_trace_")
        kwargs = dict(trace=True, tmpdir=LAST_TRACE_DIR)
    res = run_bass_kernel_spmd(nc, in_maps, list(range(NCORES)), **kwargs)
    LAST_EXEC_NS = res.exec_time_ns
    shards = [res.results[i]["out"].reshape(BS, F, M) for i in range(NCORES)]
    return np.concatenate(shards, axis=0)
